# revision 1
# baseline (speedup 1.0000x reference)
import os
import sys

sys.path.insert(0, "/opt/trn_rl_repo")
os.environ.setdefault("MYCRO_LOCAL_CACHE", "1")

import numpy as np
import ml_dtypes

BF16 = ml_dtypes.bfloat16

N_CORES = 8
P = 128  # partition / tile size
NB = int(os.environ.get("KERNEL_NB", "7"))  # gather banks == AllGathers/layer
LATE_AG = os.environ.get("KERNEL_LATE_AG", "0") == "1"  # AGs after phase, not interleaved
BARRIER = os.environ.get("KERNEL_BARRIER", "0") == "1"  # all-engine barrier after AGs
FAKE_AG = os.environ.get("KERNEL_FAKE_AG", "0") == "1"  # sim-only: AG -> local DMAs

last_exec_time_ns = None


def _preprocess(rows, cols, vals, per_core, n_tiles, group_tiles, rows_cb):
    """Segment-contiguous slot layout with dummy-edge equalization.

    Edges bucketed per (core, group g, bank b) segment, sorted by tile
    within the segment, filled contiguously into slots s -> (p, j) =
    (s % 128, base_col(g,b)*128 + s // 128).  Per-core counts equalized to
    cnt_max(g,b) so one dma_gather with num_idxs = cnt_max is SPMD-uniform
    (pad slots: idx=0/val=0 appended at the segment end, assigned to the
    group's last tile).  Columns may span tile boundaries; each (tile,
    column) pair has its own val/dst mask vectors (val=0 for slots of
    other tiles), so a shared column feeds both tiles' matmuls.

    Bank b = one AllGather of each core's h rows [b*rows_cb, (b+1)*rows_cb);
    node (core c, slice row r) lives in bank b at position c*rows_cb + r.
    """
    E = rows.shape[0]
    n_groups = -(-n_tiles // group_tiles)
    core = (rows // per_core).astype(np.int64)
    loc = rows - core * per_core
    t_loc = loc // P
    d_loc = (loc - t_loc * P).astype(np.float32)
    g_loc = t_loc // group_tiles
    t_in_g = t_loc - g_loc * group_tiles

    cc = cols // per_core
    rem = cols - cc * per_core
    b_src = rem // rows_cb
    pos = (cc * rows_cb + (rem - b_src * rows_cb)).astype(np.int16)

    # sort edges by (core, g, b, ti) -> contiguous segments, tile-major
    key = ((core * n_groups + g_loc) * NB + b_src) * group_tiles + t_in_g
    nkeys = N_CORES * n_groups * NB * group_tiles
    order = np.argsort(key, kind="stable")
    counts = np.bincount(key, minlength=nkeys)
    cnt4 = counts.reshape(N_CORES, n_groups, NB, group_tiles)
    seg_cnt = cnt4.sum(axis=3)               # [C, G, B] per-core seg len
    cnt_max = seg_cnt.max(axis=0)            # [G, B]
    Kseg = -(-cnt_max // P)                  # columns per segment
    assert cnt_max.max() <= 16 * P, cnt_max.max()

    # per-core per-segment tile boundaries (prefix over ti), then union
    pref = np.cumsum(cnt4, axis=3)           # [C, G, B, T] inclusive
    lo = np.concatenate([np.zeros_like(pref[..., :1]), pref[..., :-1]],
                        axis=3)              # [C, G, B, T] tile start slot
    hi = pref.copy()                         # tile end slot
    lo_min = lo.min(axis=0)                  # [G, B, T]
    hi_max = hi.max(axis=0)

    # column layout: group-major, bank within group
    col_of = np.zeros((n_groups, NB), dtype=np.int64)
    grp_base = np.zeros(n_groups + 1, dtype=np.int64)
    seg = np.zeros((n_groups, NB, 2), dtype=np.int64)  # (start_col, ncols)
    run = 0
    for g in range(n_groups):
        grp_base[g] = run
        for b in range(NB):
            col_of[g, b] = run
            seg[g, b, 0] = run
            seg[g, b, 1] = int(Kseg[g, b])
            run += int(Kseg[g, b])
    grp_base[n_groups] = run
    TOT = int(run)

    # (tile, column) pairs + per-tile pair lists
    pair_of = {}
    tile_pairs = [[] for _ in range(n_tiles)]
    npair = 0
    for g in range(n_groups):
        for b in range(NB):
            for ti in range(group_tiles):
                t = g * group_tiles + ti
                if t >= n_tiles:
                    continue
                c0 = int(lo_min[g, b, ti]) // P
                c1 = -(-int(hi_max[g, b, ti]) // P)
                for j in range(c0, c1):
                    jj = int(col_of[g, b]) + j
                    pair_of[(t, jj)] = npair
                    tile_pairs[t].append((npair, jj))
                    npair += 1
    for t in range(n_tiles):
        if not tile_pairs[t]:
            g = t // group_tiles
            jj = int(col_of[g, 0])
            pair_of[(t, jj)] = npair
            tile_pairs[t].append((npair, jj))
            npair += 1
    NPAIR = npair

    # slot assignment per core
    key_s = key[order]
    seg_key = key_s // group_tiles           # (core, g, b) segment id
    seg_start = np.concatenate(
        [[0], np.cumsum(counts.reshape(-1, group_tiles).sum(axis=1))])
    ranks = np.arange(E, dtype=np.int64) - seg_start[seg_key]
    cs = core[order]
    gs = g_loc[order]
    bs = b_src[order]
    ts_ = (gs * group_tiles + t_in_g[order])
    p = ranks % P
    colj = col_of[gs, bs] + ranks // P

    idx_flat = np.full((N_CORES, TOT * P), -1, dtype=np.int16)
    idx_flat[cs, colj * P + p] = pos[order]
    # per-core true count per (g,b) gather call; >=1 so the DGE never sees
    # an all-negative call (slot 0 kept as idx=0/val=0 in that case)
    cnt_tab = seg_cnt.astype(np.int32).reshape(N_CORES, -1)  # [C, G*B]
    kseg_cols = Kseg.reshape(-1)  # [G*B]
    for c in range(N_CORES):
        for si in range(cnt_tab.shape[1]):
            if kseg_cols[si] > 0 and cnt_tab[c, si] == 0:
                g, b = si // NB, si % NB
                s0 = int(col_of[g, b]) * P
                idx_flat[c, s0] = 0
                cnt_tab[c, si] = 1
    val_a = np.zeros((N_CORES, P, NPAIR), dtype=np.float32)
    dst_a = np.zeros((N_CORES, P, NPAIR), dtype=np.float32)
    parr = np.full((n_tiles, TOT), -1, dtype=np.int64)
    for (t, jj), pi in pair_of.items():
        parr[t, jj] = pi
    pidx = parr[ts_, colj]
    assert (pidx >= 0).all()
    val_a[cs, p, pidx] = vals[order]
    dst_a[cs, p, pidx] = d_loc[order]

    # dma_gather index layout: flat i at partition i%16, col i//16,
    # replicated across the 8 groups of 16 partitions.
    idx16 = np.empty((N_CORES, P, TOT * 8), dtype=np.int16)
    for c in range(N_CORES):
        a = idx_flat[c].reshape(-1, 16).T
        idx16[c] = np.tile(a, (8, 1))

    layout = dict(n_groups=n_groups, TOT=TOT, NPAIR=NPAIR,
                  grp_base=grp_base, seg=seg, cnt_max=cnt_max,
                  tile_pairs=tile_pairs, idx_flat=idx_flat, cnt_tab=cnt_tab)
    return idx16, val_a, dst_a, layout


def _build_program(F1, F2, per_core, n_tiles, group_tiles, rows_cb, layout):
    import concourse.bass as bass
    import concourse.bacc as bacc
    import concourse.mybir as mybir
    import concourse.tile as tile

    fp32 = mybir.dt.float32
    bf16 = mybir.dt.bfloat16
    i16 = mybir.dt.int16
    AF = mybir.ActivationFunctionType
    OP = mybir.AluOpType

    TOT = layout["TOT"]
    NPAIR = layout["NPAIR"]
    grp_base = layout["grp_base"]
    Lmax = max(int(grp_base[g + 1] - grp_base[g])
               for g in range(layout["n_groups"]))
    seg = layout["seg"]
    cnt_max = layout["cnt_max"]
    tile_pairs = layout["tile_pairs"]
    n_groups = layout["n_groups"]
    bank_rows = N_CORES * rows_cb
    # phase-A/C tile index after which bank b's AllGather can be issued
    issue_after = [-(-(b + 1) * rows_cb // P) - 1 for b in range(NB)]

    nc = bacc.Bacc("TRN2", target_bir_lowering=False, debug=False,
                   num_devices=N_CORES)
    xT_ext = nc.dram_tensor("xT", [F1, per_core], bf16, kind="ExternalInput")
    w1_ext = nc.dram_tensor("w1", [F1, F1], bf16, kind="ExternalInput")
    b1_ext = nc.dram_tensor("b1", [1, F1], bf16, kind="ExternalInput")
    w2_ext = nc.dram_tensor("w2", [F1, P], bf16, kind="ExternalInput")
    b2_ext = nc.dram_tensor("b2", [1, P], bf16, kind="ExternalInput")
    idx16_ext = nc.dram_tensor("idx16", [P, TOT * 8], i16,
                               kind="ExternalInput")
    val_ext = nc.dram_tensor("val", [P, NPAIR], fp32, kind="ExternalInput")
    dst_ext = nc.dram_tensor("dst", [P, NPAIR], fp32, kind="ExternalInput")
    iotaf_ext = nc.dram_tensor("iotaf", [P, P], bf16, kind="ExternalInput")
    nseg = n_groups * NB
    cnt_ext = nc.dram_tensor("cnt", [1, nseg], mybir.dt.int32,
                             kind="ExternalInput")
    out_ext = nc.dram_tensor("out", [per_core, F2], fp32,
                             kind="ExternalOutput")

    import contextlib
    rstack = contextlib.ExitStack()
    with tile.TileContext(nc) as tc:
        with tc.tile_pool(name="static", bufs=1) as static, \
             tc.tile_pool(name="dram", bufs=1, space="DRAM") as dram:
            w1_sb = static.tile([F1, F1], bf16)
            nc.sync.dma_start(w1_sb[:], w1_ext[:])
            b1_sb = static.tile([1, F1], bf16)
            nc.sync.dma_start(b1_sb[:], b1_ext[:])
            w2_sb = static.tile([F1, P], bf16)
            nc.sync.dma_start(w2_sb[:], w2_ext[:])
            b2_sb = static.tile([1, P], bf16)
            nc.sync.dma_start(b2_sb[:], b2_ext[:])
            idx16_sb = static.tile([P, TOT * 8], i16)
            nc.sync.dma_start(idx16_sb[:], idx16_ext[:])
            val_sb = static.tile([P, NPAIR], fp32)
            nc.sync.dma_start(val_sb[:], val_ext[:])
            dst_sb = static.tile([P, NPAIR], fp32)
            nc.sync.dma_start(dst_sb[:], dst_ext[:])
            ones_sb = static.tile([1, P], bf16)
            nc.vector.memset(ones_sb[:], 1.0)
            # iota from host: avoids InstIota (library 0) which deadlocks on
            # HW when interleaved with dma_gather (lib 3).
            iota_f = static.tile([P, P], bf16)
            nc.sync.dma_start(iota_f[:], iotaf_ext[:])
            xT_sb = static.tile([F1, per_core], bf16)
            nc.sync.dma_start(xT_sb[:], xT_ext[:])
            cnt_sb = static.tile([1, nseg], mybir.dt.int32)
            nc.sync.dma_start(cnt_sb[:], cnt_ext[:])

            gregs = [rstack.enter_context(
                nc.gpsimd.register(name=f"gcnt{i}")) for i in range(8)]
            gcall = [0]
            h_dram = dram.tile([per_core, F1], bf16)
            y2_dram = dram.tile([per_core, P], bf16)
            shared = "Local" if FAKE_AG else "Shared"
            H_ch = [dram.tile([bank_rows, F1], bf16, addr_space=shared,
                              name=f"H_ch{b}") for b in range(NB)]
            Y2_ch = [dram.tile([bank_rows, P], bf16, addr_space=shared,
                               name=f"Y2_ch{b}") for b in range(NB)]

            def allgather(src_dram, dst_banks, b):
                if FAKE_AG:
                    # sim-only stand-in: same bytes written, same deps
                    for c in range(N_CORES):
                        nc.sync.dma_start(
                            dst_banks[b][c * rows_cb:(c + 1) * rows_cb, :],
                            src_dram[b * rows_cb:(b + 1) * rows_cb, :])
                    return
                nc.gpsimd.collective_compute(
                    "AllGather", OP.bypass,
                    ins=[src_dram[b * rows_cb:(b + 1) * rows_cb, :]],
                    outs=[dst_banks[b][:]],
                    replica_groups=[list(range(N_CORES))])

            # ---- Phase A: h = x @ W1 + b1; allgather chunk as it completes
            tpc = rows_cb // P  # tiles per chunk/bank (14)
            with tc.tile_pool(name="ha", bufs=3) as ha, \
                 tc.tile_pool(name="psA", bufs=2,
                              space=bass.MemorySpace.PSUM) as psA:
                for t in range(n_tiles):
                    h_ps = psA.tile([P, F1], fp32)
                    nc.tensor.matmul(h_ps[:], ones_sb[:], b1_sb[:],
                                     start=True, stop=False)
                    nc.tensor.matmul(h_ps[:], xT_sb[:, t * P:(t + 1) * P],
                                     w1_sb[:], start=False, stop=True)
                    hs = t % tpc
                    if hs == 0:
                        h_stage = ha.tile([P, tpc, F1], bf16)
                    nc.scalar.activation(h_stage[:, hs, :], h_ps[:], AF.Copy)
                    if hs == tpc - 1:
                        b = t // tpc
                        nc.scalar.dma_start(
                            h_dram[b * rows_cb:(b + 1) * rows_cb, :]
                            .rearrange("(t p) f -> p t f", p=P),
                            h_stage[:])
                        allgather(h_dram, H_ch, b)

            # ---- Phase C: zT = relu(segsum L1)^T; y2 = z @ W2 + b2;
            #      allgather y2 chunk as it completes
            with tc.tile_pool(name="m1", bufs=2) as m1, \
                 tc.tile_pool(name="s1", bufs=4) as s1, \
                 tc.tile_pool(name="o1", bufs=3) as o1, \
                 tc.tile_pool(name="psZ", bufs=2,
                              space=bass.MemorySpace.PSUM) as psZ, \
                 tc.tile_pool(name="psY", bufs=2,
                              space=bass.MemorySpace.PSUM) as psY:
                for g in range(n_groups):
                    base = int(grp_base[g])
                    m_sb = m1.tile([P, Lmax, F1], bf16)
                    if g < 2:
                        nc.vector.memset(m_sb[:], 0.0)
                    for b in range(NB):
                        sA = int(seg[g, b, 0])
                        LA = int(seg[g, b, 1])
                        if LA == 0:
                            continue
                        # one call per (g,b): <= 2048 idxs fits the
                        # 128-entry SWDGE descriptor ring; num_idxs kept a
                        # multiple of 128 (HW ucode constraint).  The
                        # register holds this core's true edge count; the
                        # trailing idx=-1 pad slots are skipped by the DGE.
                        r = gregs[gcall[0] % 8]
                        gcall[0] += 1
                        si = g * NB + b
                        nc.reg_load(r, cnt_sb[0:1, si:si + 1])
                        nc.gpsimd.dma_gather(
                            m_sb[:, sA - base:sA - base + LA, :],
                            H_ch[b][:],
                            idx16_sb[:, sA * 8:(sA + LA) * 8],
                            LA * P, r, F1)
                    for t in range(g * group_tiles,
                                   min((g + 1) * group_tiles, n_tiles)):
                        cl = tile_pairs[t]
                        zT_ps = psZ.tile([F1, P], fp32)
                        for i, (pj, j) in enumerate(cl):
                            s_sb = s1.tile([P, P], bf16)
                            nc.vector.tensor_scalar(
                                out=s_sb[:], in0=iota_f[:],
                                scalar1=dst_sb[:, pj:pj + 1],
                                scalar2=val_sb[:, pj:pj + 1],
                                op0=OP.is_equal, op1=OP.mult)
                            nc.tensor.matmul(zT_ps[:], m_sb[:, j - base, :],
                                             s_sb[:],
                                             start=(i == 0),
                                             stop=(i == len(cl) - 1))
                        zT_sb = o1.tile([F1, P], bf16)
                        nc.scalar.activation(zT_sb[:], zT_ps[:], AF.Relu)
                        y2_ps = psY.tile([P, P], fp32)
                        nc.tensor.matmul(y2_ps[:], ones_sb[:], b2_sb[:],
                                         start=True, stop=False)
                        nc.tensor.matmul(y2_ps[:], zT_sb[:], w2_sb[:],
                                         start=False, stop=True)
                        ys = t % tpc
                        if ys == 0:
                            y2_stage = o1.tile([P, tpc, P], bf16)
                        nc.scalar.activation(y2_stage[:, ys, :], y2_ps[:],
                                             AF.Copy)
                        if ys == tpc - 1:
                            b = t // tpc
                            nc.scalar.dma_start(
                                y2_dram[b * rows_cb:(b + 1) * rows_cb, :]
                                .rearrange("(t p) f -> p t f", p=P),
                                y2_stage[:])
                            allgather(y2_dram, Y2_ch, b)

            # ---- Phase E: out = segsum L2 ----
            with tc.tile_pool(name="m2", bufs=2) as m2, \
                 tc.tile_pool(name="s2", bufs=4) as s2, \
                 tc.tile_pool(name="o2", bufs=2) as o2, \
                 tc.tile_pool(name="psO", bufs=2,
                              space=bass.MemorySpace.PSUM) as psO:
                for g in range(n_groups):
                    base = int(grp_base[g])
                    m_sb = m2.tile([P, Lmax, P], bf16)
                    if g < 2:
                        nc.vector.memset(m_sb[:], 0.0)
                    for b in range(NB):
                        sA = int(seg[g, b, 0])
                        LA = int(seg[g, b, 1])
                        if LA == 0:
                            continue
                        r = gregs[gcall[0] % 8]
                        gcall[0] += 1
                        si = g * NB + b
                        nc.reg_load(r, cnt_sb[0:1, si:si + 1])
                        nc.gpsimd.dma_gather(
                            m_sb[:, sA - base:sA - base + LA, :],
                            Y2_ch[b][:],
                            idx16_sb[:, sA * 8:(sA + LA) * 8],
                            LA * P, r, P)
                    for t in range(g * group_tiles,
                                   min((g + 1) * group_tiles, n_tiles)):
                        cl = tile_pairs[t]
                        o_ps = psO.tile([P, F2], fp32)
                        for i, (pj, j) in enumerate(cl):
                            s_sb = s2.tile([P, P], bf16)
                            nc.vector.tensor_scalar(
                                out=s_sb[:], in0=iota_f[:],
                                scalar1=dst_sb[:, pj:pj + 1],
                                scalar2=val_sb[:, pj:pj + 1],
                                op0=OP.is_equal, op1=OP.mult)
                            nc.tensor.matmul(o_ps[:], s_sb[:],
                                             m_sb[:, j - base, 0:F2],
                                             start=(i == 0),
                                             stop=(i == len(cl) - 1))
                        os_ = t % tpc
                        if os_ == 0:
                            o_stage = o2.tile([P, tpc, F2], fp32)
                        nc.scalar.activation(o_stage[:, os_, :], o_ps[:],
                                             AF.Copy)
                        if os_ == tpc - 1:
                            t0c = (t // tpc) * tpc
                            nc.scalar.dma_start(
                                out_ext[t0c * P:(t + 1) * P, :]
                                .rearrange("(t p) f -> p t f", p=P),
                                o_stage[:])

    rstack.close()
    nc.compile()
    return nc


def _run(rows, cols, vals, x, W1, b1, W2, b2, group_tiles=None, trace=False):
    if group_tiles is None:
        group_tiles = int(os.environ.get("KERNEL_GT", "3"))
    from concourse.bass_utils import run_bass_kernel_spmd

    n_nodes, F1 = x.shape
    F2 = W2.shape[1]
    NP_ = -(-n_nodes // (N_CORES * P)) * (N_CORES * P)
    per_core = NP_ // N_CORES
    n_tiles = per_core // P
    rows_cb = per_core // NB
    assert rows_cb * NB == per_core

    idx16, val_a, dst_a, layout = _preprocess(
        rows, cols, vals, per_core, n_tiles, group_tiles, rows_cb)

    x_pad = np.zeros((NP_, F1), dtype=np.float32)
    x_pad[:n_nodes] = x
    w2p = np.zeros((F1, P), dtype=BF16)
    w2p[:, :F2] = W2.astype(BF16)
    b2p = np.zeros((1, P), dtype=BF16)
    b2p[0, :F2] = b2.astype(BF16)

    nc = _build_program(F1, F2, per_core, n_tiles, group_tiles, rows_cb,
                        layout)

    in_maps = []
    for c in range(N_CORES):
        xTc = np.ascontiguousarray(
            x_pad[c * per_core:(c + 1) * per_core].T.astype(BF16))
        in_maps.append({
            "xT": xTc,
            "w1": np.ascontiguousarray(W1.astype(BF16)),
            "b1": np.ascontiguousarray(b1.reshape(1, F1).astype(BF16)),
            "w2": w2p,
            "b2": b2p,
            "idx16": np.ascontiguousarray(idx16[c]),
            "val": np.ascontiguousarray(val_a[c]),
            "dst": np.ascontiguousarray(dst_a[c]),
            "iotaf": np.tile(np.arange(P, dtype=np.float32).astype(BF16),
                             (P, 1)),
            "cnt": np.ascontiguousarray(layout["cnt_tab"][c:c + 1]),
        })

    import time as _time
    t0 = _time.perf_counter()
    res = run_bass_kernel_spmd(nc, in_maps, core_ids=list(range(N_CORES)),
                               trace=trace)
    wall_ns = int((_time.perf_counter() - t0) * 1e9)
    t_ns = res.exec_time_ns if res.exec_time_ns is not None else wall_ns
    out = np.concatenate([res.results[c]["out"] for c in range(N_CORES)],
                         axis=0)[:n_nodes]
    return out, t_ns


def kernel(**inputs):
    global last_exec_time_ns
    trace = os.environ.get("KERNEL_TRACE", "0") == "1"
    inputs = {k: np.asarray(v) for k, v in inputs.items()}
    out, t_ns = _run(inputs["rows"], inputs["cols"], inputs["vals"],
                     inputs["x"], inputs["W1"], inputs["b1"],
                     inputs["W2"], inputs["b2"], trace=trace)
    last_exec_time_ns = t_ns
    return out

